# revision 60
# baseline (speedup 1.0000x reference)
"""Trainium2 Bass kernel for the AdaptLoss direct-fuse loss function.

Reference computation (full [16,3,512,512] tensors gt, s_gt, t_gt and tiny
gate params convW/convB/linW/linB):
    pooled = stack([mean_hw(gt), mean_hw(t_gt)])          # [N,C,2]
    logits = MLP(pooled)  (per-channel 2->32->2 affine)   # [N,C,2]
    a, b  = softmax(logits)[..., 0], [..., 1]             # b == 1 - a
    fused = a*gt + b*t_gt
    out   = mean(|s_gt - fused|)                          # scalar

The MLP is linear up to the softmax, so it folds on the host to
    d(n,c) = alpha0[c]*mean(gt[n,c]) + alpha1[c]*mean(t_gt[n,c]) + beta[c]
    a = sigmoid(d),  b = 1 - a.

Device strategy (pure data parallel over N, 2 samples = 6 (n,c) planes/core):
  - All tensors f32r-typed so TensorE runs the fast fp32 matmul path with no
    cast or rounding pass. Two concurrent DMA rings (SP HWDGE + GPSIMD SWDGE)
    stream per-plane pieces, ordered so the last bytes to land feed the
    shortest remaining chains; gate partition-reduces slot between transfers.
  - phase 1: per-plane sums split across engines — gt on ScalarE (Copy with
    accum_out), t_gt on DVE (tensor_reduce); separate per-engine accumulator
    tiles avoid cross-engine WAW event-semaphore chains (1-wait HW limit).
  - per-plane gate: GPSIMD partition_all_reduce -> totals on every partition;
    DVE folds alpha; ScalarE sigmoid (beta as bias AP); DVE builds diagonal
    [128,128] f32r stationaries diag(a), diag(1-a).
  - phase 2 (TensorE): z = diag(a).g + diag(b).t - I.s accumulated in PSUM via
    f32r matmuls (1 cycle/row); each plane drained by ScalarE Abs+accum (half,
    in place on PSUM) and DVE abs-reduce (half) in parallel.
  - output: partition_all_reduce of the per-engine [128,6] partials -> one
    [1,12] row per core; host sums and divides by N*C*H*W.
"""

import numpy as np

N, C, H, W = 16, 3, 512, 512
NCORES = 8
NPER = N // NCORES          # samples per core
PLANES = NPER * C           # (n,c) planes per core
P = 128                     # SBUF partitions
PF = (H * W) // P           # 2048 free elems per partition per plane
HALFP = PLANES // 2         # planes per half
HF = HALFP * PF             # free elems per half tile
CHUNK = 512                 # matmul moving free dim (one PSUM bank)
HWC = float(H * W)
LOSS_WEIGHT = 1.0
NCONST = 3 * PLANES + 2 * P  # gate cols + ident + negI

_CACHE = {}


def _build_nc():
    import concourse.bacc as bacc
    import concourse.mybir as mybir
    from concourse.tile import TileContext
    from concourse import bass_isa

    f32 = mybir.dt.float32
    f32r = mybir.dt.float32r
    AF = mybir.ActivationFunctionType
    ALU = mybir.AluOpType

    nc = bacc.Bacc()
    gt_e = nc.declare_dram_parameter("gt", [2, P, HF], f32r, isOutput=False)
    tgt_e = nc.declare_dram_parameter("t_gt", [2, P, HF], f32r, isOutput=False)
    sgt_e = nc.declare_dram_parameter("s_gt", [2, P, HF], f32r, isOutput=False)
    # one merged const tensor: cols 0:18 gate (alpha0/HW, alpha1/HW, beta),
    # 18:146 identity, 146:274 -identity. f32r so the negI slice feeds matmul.
    const_e = nc.declare_dram_parameter("consts", [P, NCONST], f32r, isOutput=False)
    out_e = nc.declare_dram_parameter("out", [1, 2 * PLANES + 1], f32, isOutput=True)

    with TileContext(nc) as tc:
        with (
            tc.tile_pool(name="data", bufs=1) as data,
            tc.tile_pool(name="small", bufs=1) as small,
            tc.tile_pool(name="diag", bufs=1) as diagp,
            tc.tile_pool(name="ps", bufs=2, space="PSUM") as psp,
        ):
            # Accumulators are split per writing engine (ACT vs DVE): a tile
            # with writers on two engines forces bacc event-semaphore chains
            # (1-wait HW limit) that serialize the tail.
            # extra S_g column: plane 5's first t-chunk sum is computed by ACT
            # (parallel to DVE's second chunk) to shorten the DVE tail queue
            S_g = small.tile([P, PLANES + 1], f32, tag="S_g")    # ACT writes
            S_t = small.tile([P, PLANES + 1], f32, tag="S_t")    # DVE writes
            TT_g = small.tile([P, PLANES + 1], f32, tag="TT_g")  # Pool writes
            TT_t = small.tile([P, PLANES + 1], f32, tag="TT_t")  # Pool writes
            D1 = small.tile([P, PLANES], f32, tag="D1")
            D2 = small.tile([P, PLANES], f32, tag="D2")
            A = small.tile([P, PLANES], f32, tag="A")
            B = small.tile([P, PLANES], f32, tag="B")
            R_act = small.tile([P, PLANES], f32, tag="R_act")  # |z| partials
            R_dve = small.tile([P, PLANES + 1], f32, tag="R_dve")
            R2a = small.tile([P, PLANES], f32, tag="R2a")
            R2b = small.tile([P, PLANES + 1], f32, tag="R2b")
            scratch = small.tile([P, PF], f32, tag="scratch")

            # --- data tiles (g0/t0 split so half-0 sums start early) ---
            g0a = data.tile([P, PF], f32r, name="g0a", tag="g0a")
            g0b = data.tile([P, 2 * PF], f32r, name="g0b", tag="g0b")
            t0a = data.tile([P, PF], f32r, name="t0a", tag="t0a")
            t0b = data.tile([P, 2 * PF], f32r, name="t0b", tag="t0b")
            g1 = data.tile([P, HF], f32r, name="g1", tag="g1")
            s0p = [data.tile([P, PF], f32r, name=f"s0p{i}", tag=f"s0p{i}") for i in range(HALFP)]
            t1p = [data.tile([P, PF], f32r, name=f"t1p{i}", tag=f"t1p{i}") for i in range(2)]
            t1p2a = data.tile([P, PF // 2], f32r, name="t1p2a", tag="t1p2a")
            t1p2b = data.tile([P, PF // 2], f32r, name="t1p2b", tag="t1p2b")
            s1p = [data.tile([P, PF], f32r, name=f"s1p{i}", tag=f"s1p{i}") for i in range(HALFP)]
            consts = small.tile([P, NCONST], f32r, tag="consts")

            # --- DMA schedule: two rings run concurrently (SP HWDGE and
            # GPSIMD SWDGE). Emission order doubles as scheduler priority, so
            # s pieces and PARs are emitted interleaved to share Pool's ring.
            nc.sync.dma_start(out=g0a[:], in_=gt_e[0, :, 0:PF])
            nc.gpsimd.dma_start(out=t0a[:], in_=tgt_e[0, :, 0:PF])
            nc.sync.dma_start(out=g0b[:], in_=gt_e[0, :, PF : 3 * PF])
            nc.gpsimd.dma_start(out=t0b[:], in_=tgt_e[0, :, PF : 3 * PF])
            nc.sync.dma_start(out=consts[:], in_=const_e[:])
            nc.gpsimd.dma_start(out=s0p[0][:], in_=sgt_e[0, :, 0:PF])
            nc.gpsimd.dma_start(out=s0p[1][:], in_=sgt_e[0, :, PF : 2 * PF])
            nc.gpsimd.dma_start(out=s0p[2][:], in_=sgt_e[0, :, 2 * PF : 3 * PF])
            nc.sync.dma_start(out=g1[:], in_=gt_e[1])

            gate_f = consts[:, 0 : 3 * PLANES].bitcast(f32)
            ident_f = consts[:, 3 * PLANES : 3 * PLANES + P].bitcast(f32)
            negi_r = consts[:, 3 * PLANES + P : NCONST]

            diagA = [diagp.tile([P, P], f32r, name=f"dA{p}", tag=f"dA{p}") for p in range(PLANES)]
            diagB = [diagp.tile([P, P], f32r, name=f"dB{p}", tag=f"dB{p}") for p in range(PLANES)]

            def g_sl(p):
                if p == 0:
                    return g0a[:]
                if p < HALFP:
                    return g0b[:, (p - 1) * PF : p * PF]
                return g1[:, (p % HALFP) * PF : (p % HALFP + 1) * PF]

            def t_sl(p):
                if p == 0:
                    return t0a[:]
                if p < HALFP:
                    return t0b[:, (p - 1) * PF : p * PF]
                return t1p[p - HALFP][:]

            def t_chunk(p, ci):
                if p == PLANES - 1:
                    tile = t1p2a if ci < 2 else t1p2b
                    return tile[:, (ci % 2) * CHUNK : (ci % 2 + 1) * CHUNK]
                return t_sl(p)[:, ci * CHUNK : (ci + 1) * CHUNK]

            def s_sl(p):
                if p < HALFP:
                    return s0p[p][:]
                return s1p[p - HALFP][:]

            # Per-plane pipeline pieces. Emission order is hand-scheduled so
            # that no in-order engine stream ever has a late-dependency op
            # (drain, t-sums) in front of an earlier-ready one.
            zps = [None] * PLANES

            def ph1_g(p):
                nc.scalar.activation(
                    scratch[:], g_sl(p).bitcast(f32), AF.Copy,
                    accum_out=S_g[:, p : p + 1],
                )

            def ph1_t(p):
                # t sums on DVE so phase-1 splits across both engines
                if p == PLANES - 1:
                    # chunk a on ACT, chunk b on DVE: both engines finish the
                    # last t-sum in parallel ~1.2us after the bytes land
                    nc.scalar.activation(
                        scratch[:, 0 : PF // 2], t1p2a[:].bitcast(f32), AF.Copy,
                        accum_out=S_g[:, PLANES : PLANES + 1],
                    )
                    nc.vector.tensor_reduce(
                        S_t[:, p : p + 1], t1p2b[:].bitcast(f32),
                        mybir.AxisListType.X, ALU.add,
                    )
                    return
                nc.vector.tensor_reduce(
                    S_t[:, p : p + 1], t_sl(p).bitcast(f32),
                    mybir.AxisListType.X, ALU.add,
                )

            def gate(p):
                # totals -> d -> a = sigmoid(d + beta) -> diag stationaries
                last = p == PLANES - 1
                ncols = 2 if last else 1  # last: S_g col 6 holds t5a's sum
                nc.gpsimd.partition_all_reduce(
                    TT_g[:, p : p + ncols], S_g[:, p : p + ncols],
                    channels=P, reduce_op=bass_isa.ReduceOp.add,
                )
                nc.gpsimd.partition_all_reduce(
                    TT_t[:, p : p + 1], S_t[:, p : p + 1],
                    channels=P, reduce_op=bass_isa.ReduceOp.add,
                )
                nc.vector.tensor_tensor(
                    D1[:, p : p + 1], TT_g[:, p : p + 1],
                    gate_f[:, p : p + 1], ALU.mult,
                )
                if last:
                    # plane 5's t total = ACT-chunk sum + DVE-chunk sum
                    nc.vector.tensor_tensor(
                        D2[:, p : p + 1], TT_g[:, p + 1 : p + 2],
                        TT_t[:, p : p + 1], ALU.add,
                    )
                    nc.vector.tensor_tensor(
                        D2[:, p : p + 1], D2[:, p : p + 1],
                        gate_f[:, PLANES + p : PLANES + p + 1], ALU.mult,
                    )
                else:
                    nc.vector.tensor_tensor(
                        D2[:, p : p + 1], TT_t[:, p : p + 1],
                        gate_f[:, PLANES + p : PLANES + p + 1], ALU.mult,
                    )
                nc.vector.tensor_tensor(
                    D1[:, p : p + 1], D1[:, p : p + 1], D2[:, p : p + 1], ALU.add
                )
                nc.scalar.activation(
                    A[:, p : p + 1], D1[:, p : p + 1], AF.Sigmoid,
                    bias=gate_f[:, 2 * PLANES + p : 2 * PLANES + p + 1],
                )
                nc.vector.tensor_scalar(
                    B[:, p : p + 1], A[:, p : p + 1], -1.0, 1.0, ALU.mult, ALU.add
                )
                nc.vector.tensor_scalar(
                    diagA[p][:], ident_f[:], A[:, p : p + 1], None, ALU.mult
                )
                nc.vector.tensor_scalar(
                    diagB[p][:], ident_f[:], B[:, p : p + 1], None, ALU.mult
                )

            def mms(p):
                # z = a*g + b*t - s accumulated in PSUM by TensorE.
                # Two half-plane PSUM tiles (2 banks each) so the ACT-drained
                # and DVE-drained halves recycle independently.
                za = psp.tile([P, PF // 2], f32, name=f"za{p}", tag="za")
                zb = psp.tile([P, PF // 2], f32, name=f"zb{p}", tag="zb")
                zps[p] = (za, zb)
                gs, ss = g_sl(p), s_sl(p)
                for ci in range(PF // CHUNK):
                    zp = za if ci < (PF // CHUNK) // 2 else zb
                    sl = slice((ci % 2) * CHUNK, (ci % 2) * CHUNK + CHUNK)
                    gsl = slice(ci * CHUNK, (ci + 1) * CHUNK)
                    nc.tensor.matmul(
                        zp[:, sl], diagA[p][:], gs[:, gsl], start=True, stop=False
                    )
                    nc.tensor.matmul(
                        zp[:, sl], diagB[p][:], t_chunk(p, ci), start=False, stop=False
                    )
                    nc.tensor.matmul(
                        zp[:, sl], negi_r[:], ss[:, gsl], start=False, stop=True
                    )

            def drain(p):
                za, zb = zps[p]
                # abs written back in place to PSUM: no SBUF scratch, so no
                # WAW chain against phase-1's scratch dumps on ACT
                nc.scalar.activation(
                    za[:], za[:], AF.Abs,
                    accum_out=R_act[:, p : p + 1],
                )
                if p == PLANES - 1:
                    # last plane: drain zb per chunk so the first half overlaps
                    # the final matmuls and only ~0.6us trails the last chunk
                    nc.vector.tensor_reduce(
                        R_dve[:, p : p + 1], zb[:, 0:CHUNK],
                        mybir.AxisListType.X, ALU.add, apply_absolute_value=True,
                    )
                    nc.vector.tensor_reduce(
                        R_dve[:, p + 1 : p + 2], zb[:, CHUNK : 2 * CHUNK],
                        mybir.AxisListType.X, ALU.add, apply_absolute_value=True,
                    )
                    return
                nc.vector.tensor_reduce(
                    R_dve[:, p : p + 1], zb[:],
                    mybir.AxisListType.X, ALU.add, apply_absolute_value=True,
                )

            # half 0: s0p2's DMA is emitted after gate(1) so PAR(0)/PAR(1)
            # slot into Pool's ring between the s-piece transfers.
            for p in range(HALFP):
                ph1_g(p)
                ph1_t(p)
                gate(p)
                mms(p)
            # half-1 tail pieces (after the half-0 PARs in Pool's stream):
            # t pieces ride the SP ring (their chain is long: sum->gate->mm),
            # s pieces ride the Pool ring interleaved with the half-1 PARs.
            nc.sync.dma_start(out=t1p[0][:], in_=tgt_e[1, :, 0:PF])
            nc.gpsimd.dma_start(out=s1p[0][:], in_=sgt_e[1, :, 0:PF])
            nc.sync.dma_start(out=t1p[1][:], in_=tgt_e[1, :, PF : 2 * PF])
            nc.gpsimd.dma_start(out=s1p[1][:], in_=sgt_e[1, :, PF : 2 * PF])
            nc.sync.dma_start(out=t1p2a[:], in_=tgt_e[1, :, 2 * PF : 2 * PF + PF // 2])
            nc.sync.dma_start(out=t1p2b[:], in_=tgt_e[1, :, 2 * PF + PF // 2 : 3 * PF])
            nc.gpsimd.dma_start(out=s1p[2][:], in_=sgt_e[1, :, 2 * PF : 3 * PF])
            drain(0)
            drain(1)
            # half 1: g sums hoisted (g1 lands before the t/s pieces); per-
            # plane chains follow each t piece; drains staggered behind.
            ph1_g(3)
            ph1_g(4)
            ph1_g(5)
            ph1_t(3)
            gate(3)
            mms(3)
            drain(2)
            ph1_t(4)
            gate(4)
            mms(4)
            ph1_t(5)
            gate(5)
            mms(5)
            drain(3)
            drain(4)
            drain(5)

            # collapse partitions so the output DMA is one descriptor
            nc.gpsimd.partition_all_reduce(
                R2a[:], R_act[:], channels=P, reduce_op=bass_isa.ReduceOp.add
            )
            nc.gpsimd.partition_all_reduce(
                R2b[:], R_dve[:], channels=P, reduce_op=bass_isa.ReduceOp.add
            )
            # two rings so the final two tiny DMAs run concurrently
            nc.sync.dma_start(out=out_e[0:1, 0:PLANES], in_=R2a[0:1, :])
            nc.gpsimd.dma_start(
                out=out_e[0:1, PLANES : 2 * PLANES + 1], in_=R2b[0:1, :]
            )

    nc.finalize()
    return nc


def _gate_matrix(convW, convB, linW, linB):
    """Fold the per-channel 2->32->2 MLP + softmax-diff into d = alpha.pooled+beta."""
    w = (linW[:, 0, :] - linW[:, 1, :]).astype(np.float64)        # [3,32]
    alpha = np.einsum("co,coj->cj", w, convW.astype(np.float64))  # [3,2]
    beta = (w * convB.astype(np.float64)).sum(1) + (
        linB[:, 0].astype(np.float64) - linB[:, 1].astype(np.float64)
    )                                                             # [3]
    row = np.zeros(3 * PLANES, dtype=np.float64)
    for p in range(PLANES):
        c = p % C
        row[p] = alpha[c, 0] / HWC
        row[PLANES + p] = alpha[c, 1] / HWC
        row[2 * PLANES + p] = beta[c]
    return row.astype(np.float32)


def _make_in_maps(inputs):
    gate_row = _gate_matrix(
        np.asarray(inputs["convW"], dtype=np.float32),
        np.asarray(inputs["convB"], dtype=np.float32),
        np.asarray(inputs["linW"], dtype=np.float32),
        np.asarray(inputs["linB"], dtype=np.float32),
    )
    consts = np.zeros((P, NCONST), dtype=np.float32)
    consts[:, 0 : 3 * PLANES] = gate_row[None, :]
    consts[:, 3 * PLANES : 3 * PLANES + P] = np.eye(P, dtype=np.float32)
    consts[:, 3 * PLANES + P : NCONST] = -np.eye(P, dtype=np.float32)

    def shards(x):
        x = np.asarray(x, dtype=np.float32)
        # [16,3,512,512] -> per core [2 halves, 128, 3*2048]
        x = x.reshape(NCORES, 2, HALFP, P, PF)
        x = x.transpose(0, 1, 3, 2, 4)  # [cores, half, P, planes, PF]
        return np.ascontiguousarray(x.reshape(NCORES, 2, P, HF))

    g_s, t_s, s_s = shards(inputs["gt"]), shards(inputs["t_gt"]), shards(inputs["s_gt"])
    return [
        {"gt": g_s[i], "t_gt": t_s[i], "s_gt": s_s[i], "consts": consts}
        for i in range(NCORES)
    ]


def _run(inputs, trace=False):
    import time

    from concourse.bass_utils import run_bass_kernel_spmd

    if "nc" not in _CACHE:
        _CACHE["nc"] = _build_nc()
    nc = _CACHE["nc"]

    in_maps = _make_in_maps(inputs)
    res = None
    for attempt in range(3):
        try:
            res = run_bass_kernel_spmd(nc, in_maps, list(range(NCORES)), trace=trace)
            break
        except Exception:
            # first execution of a freshly compiled NEFF occasionally hits a
            # transient device error on this fleet; retry
            if attempt == 2:
                raise
            time.sleep(10)
    total = np.float64(0.0)
    for i in range(NCORES):
        total += np.asarray(res.results[i]["out"], dtype=np.float64).sum()
    mean = total / float(N * C * H * W)
    return np.float32(LOSS_WEIGHT * mean), res


def kernel(**inputs) -> np.ndarray:
    out, _ = _run(inputs, trace=False)
    return out


# revision 61
# speedup vs baseline: 1.0384x; 1.0384x over previous
"""Trainium2 Bass kernel for the AdaptLoss direct-fuse loss function.

Reference computation (full [16,3,512,512] tensors gt, s_gt, t_gt and tiny
gate params convW/convB/linW/linB):
    pooled = stack([mean_hw(gt), mean_hw(t_gt)])          # [N,C,2]
    logits = MLP(pooled)  (per-channel 2->32->2 affine)   # [N,C,2]
    a, b  = softmax(logits)[..., 0], [..., 1]             # b == 1 - a
    fused = a*gt + b*t_gt
    out   = mean(|s_gt - fused|)                          # scalar

The MLP is linear up to the softmax, so it folds on the host to
    d(n,c) = alpha0[c]*mean(gt[n,c]) + alpha1[c]*mean(t_gt[n,c]) + beta[c]
    a = sigmoid(d),  b = 1 - a.

Device strategy (pure data parallel over N, 2 samples = 6 (n,c) planes/core):
  - All tensors f32r-typed so TensorE runs the fast fp32 matmul path with no
    cast or rounding pass. Two concurrent DMA rings (SP HWDGE + GPSIMD SWDGE)
    stream per-plane pieces, ordered so the last bytes to land feed the
    shortest remaining chains; gate partition-reduces slot between transfers.
  - phase 1: per-plane sums split across engines — gt on ScalarE (Copy with
    accum_out), t_gt on DVE (tensor_reduce); separate per-engine accumulator
    tiles avoid cross-engine WAW event-semaphore chains (1-wait HW limit).
  - per-plane gate: GPSIMD partition_all_reduce -> totals on every partition;
    DVE folds alpha; ScalarE sigmoid (beta as bias AP); DVE builds diagonal
    [128,128] f32r stationaries diag(a), diag(1-a).
  - phase 2 (TensorE): z = diag(a).g + diag(b).t - I.s accumulated in PSUM via
    f32r matmuls (1 cycle/row); each plane drained by ScalarE Abs+accum (half,
    in place on PSUM) and DVE abs-reduce (half) in parallel.
  - output: partition_all_reduce of the per-engine [128,6] partials -> one
    [1,12] row per core; host sums and divides by N*C*H*W.
"""

import numpy as np

N, C, H, W = 16, 3, 512, 512
NCORES = 8
NPER = N // NCORES          # samples per core
PLANES = NPER * C           # (n,c) planes per core
P = 128                     # SBUF partitions
PF = (H * W) // P           # 2048 free elems per partition per plane
HALFP = PLANES // 2         # planes per half
HF = HALFP * PF             # free elems per half tile
CHUNK = 512                 # matmul moving free dim (one PSUM bank)
HWC = float(H * W)
LOSS_WEIGHT = 1.0
NCONST = 3 * PLANES + 2 * P  # gate cols + ident + negI

_CACHE = {}


def _build_nc():
    import concourse.bacc as bacc
    import concourse.mybir as mybir
    from concourse.tile import TileContext
    from concourse import bass_isa

    f32 = mybir.dt.float32
    f32r = mybir.dt.float32r
    AF = mybir.ActivationFunctionType
    ALU = mybir.AluOpType

    nc = bacc.Bacc()
    gt_e = nc.declare_dram_parameter("gt", [2, P, HF], f32r, isOutput=False)
    tgt_e = nc.declare_dram_parameter("t_gt", [2, P, HF], f32r, isOutput=False)
    sgt_e = nc.declare_dram_parameter("s_gt", [2, P, HF], f32r, isOutput=False)
    # one merged const tensor: cols 0:18 gate (alpha0/HW, alpha1/HW, beta),
    # 18:146 identity, 146:274 -identity. f32r so the negI slice feeds matmul.
    const_e = nc.declare_dram_parameter("consts", [P, NCONST], f32r, isOutput=False)
    out_e = nc.declare_dram_parameter("out", [1, 2 * PLANES], f32, isOutput=True)

    with TileContext(nc) as tc:
        with (
            tc.tile_pool(name="data", bufs=1) as data,
            tc.tile_pool(name="small", bufs=1) as small,
            tc.tile_pool(name="diag", bufs=1) as diagp,
            tc.tile_pool(name="ps", bufs=2, space="PSUM") as psp,
        ):
            # Accumulators are split per writing engine (ACT vs DVE): a tile
            # with writers on two engines forces bacc event-semaphore chains
            # (1-wait HW limit) that serialize the tail.
            S_g = small.tile([P, PLANES], f32, tag="S_g")        # ACT writes
            S_t = small.tile([P, PLANES + 1], f32, tag="S_t")    # DVE writes
            TT_g = small.tile([P, PLANES], f32, tag="TT_g")      # Pool writes
            TT_t = small.tile([P, PLANES + 1], f32, tag="TT_t")  # Pool writes
            D1 = small.tile([P, PLANES], f32, tag="D1")
            D2 = small.tile([P, PLANES], f32, tag="D2")
            A = small.tile([P, PLANES], f32, tag="A")
            B = small.tile([P, PLANES], f32, tag="B")
            R_act = small.tile([P, PLANES], f32, tag="R_act")  # |z| partials
            R_dve = small.tile([P, PLANES], f32, tag="R_dve")
            R2a = small.tile([P, PLANES], f32, tag="R2a")
            R2b = small.tile([P, PLANES], f32, tag="R2b")
            scratch = small.tile([P, PF], f32, tag="scratch")

            # --- data tiles (g0/t0 split so half-0 sums start early) ---
            g0a = data.tile([P, PF], f32r, name="g0a", tag="g0a")
            g0b = data.tile([P, 2 * PF], f32r, name="g0b", tag="g0b")
            t0a = data.tile([P, PF], f32r, name="t0a", tag="t0a")
            t0b = data.tile([P, 2 * PF], f32r, name="t0b", tag="t0b")
            g1 = data.tile([P, HF], f32r, name="g1", tag="g1")
            s0p = [data.tile([P, PF], f32r, name=f"s0p{i}", tag=f"s0p{i}") for i in range(HALFP)]
            t1p = [data.tile([P, PF], f32r, name=f"t1p{i}", tag=f"t1p{i}") for i in range(2)]
            t1p2a = data.tile([P, PF // 2], f32r, name="t1p2a", tag="t1p2a")
            t1p2b = data.tile([P, PF // 2], f32r, name="t1p2b", tag="t1p2b")
            s1p = [data.tile([P, PF], f32r, name=f"s1p{i}", tag=f"s1p{i}") for i in range(HALFP)]
            consts = small.tile([P, NCONST], f32r, tag="consts")

            # --- DMA schedule: two rings run concurrently (SP HWDGE and
            # GPSIMD SWDGE). Emission order doubles as scheduler priority, so
            # s pieces and PARs are emitted interleaved to share Pool's ring.
            nc.sync.dma_start(out=g0a[:], in_=gt_e[0, :, 0:PF])
            nc.gpsimd.dma_start(out=t0a[:], in_=tgt_e[0, :, 0:PF])
            nc.sync.dma_start(out=g0b[:], in_=gt_e[0, :, PF : 3 * PF])
            nc.gpsimd.dma_start(out=t0b[:], in_=tgt_e[0, :, PF : 3 * PF])
            nc.sync.dma_start(out=consts[:], in_=const_e[:])
            nc.gpsimd.dma_start(out=s0p[0][:], in_=sgt_e[0, :, 0:PF])
            nc.gpsimd.dma_start(out=s0p[1][:], in_=sgt_e[0, :, PF : 2 * PF])
            nc.gpsimd.dma_start(out=s0p[2][:], in_=sgt_e[0, :, 2 * PF : 3 * PF])
            nc.sync.dma_start(out=g1[:], in_=gt_e[1])

            gate_f = consts[:, 0 : 3 * PLANES].bitcast(f32)
            ident_f = consts[:, 3 * PLANES : 3 * PLANES + P].bitcast(f32)
            negi_r = consts[:, 3 * PLANES + P : NCONST]

            diagA = [diagp.tile([P, P], f32r, name=f"dA{p}", tag=f"dA{p}") for p in range(PLANES)]
            diagB = [diagp.tile([P, P], f32r, name=f"dB{p}", tag=f"dB{p}") for p in range(PLANES)]

            def g_sl(p):
                if p == 0:
                    return g0a[:]
                if p < HALFP:
                    return g0b[:, (p - 1) * PF : p * PF]
                return g1[:, (p % HALFP) * PF : (p % HALFP + 1) * PF]

            def t_sl(p):
                if p == 0:
                    return t0a[:]
                if p < HALFP:
                    return t0b[:, (p - 1) * PF : p * PF]
                return t1p[p - HALFP][:]

            def t_chunk(p, ci):
                if p == PLANES - 1:
                    tile = t1p2a if ci < 2 else t1p2b
                    return tile[:, (ci % 2) * CHUNK : (ci % 2 + 1) * CHUNK]
                return t_sl(p)[:, ci * CHUNK : (ci + 1) * CHUNK]

            def s_sl(p):
                if p < HALFP:
                    return s0p[p][:]
                return s1p[p - HALFP][:]

            # Per-plane pipeline pieces. Emission order is hand-scheduled so
            # that no in-order engine stream ever has a late-dependency op
            # (drain, t-sums) in front of an earlier-ready one.
            zps = [None] * PLANES

            def ph1_g(p):
                nc.scalar.activation(
                    scratch[:], g_sl(p).bitcast(f32), AF.Copy,
                    accum_out=S_g[:, p : p + 1],
                )

            def ph1_t(p):
                # t sums on DVE so phase-1 splits across both engines
                if p == PLANES - 1:
                    nc.vector.tensor_reduce(
                        S_t[:, p : p + 1], t1p2a[:].bitcast(f32),
                        mybir.AxisListType.X, ALU.add,
                    )
                    nc.vector.tensor_reduce(
                        S_t[:, p + 1 : p + 2], t1p2b[:].bitcast(f32),
                        mybir.AxisListType.X, ALU.add,
                    )
                    return
                nc.vector.tensor_reduce(
                    S_t[:, p : p + 1], t_sl(p).bitcast(f32),
                    mybir.AxisListType.X, ALU.add,
                )

            def gate(p):
                # totals -> d -> a = sigmoid(d + beta) -> diag stationaries
                last = p == PLANES - 1
                nc.gpsimd.partition_all_reduce(
                    TT_g[:, p : p + 1], S_g[:, p : p + 1],
                    channels=P, reduce_op=bass_isa.ReduceOp.add,
                )
                ncols = 2 if last else 1
                nc.gpsimd.partition_all_reduce(
                    TT_t[:, p : p + ncols], S_t[:, p : p + ncols],
                    channels=P, reduce_op=bass_isa.ReduceOp.add,
                )
                nc.vector.tensor_tensor(
                    D1[:, p : p + 1], TT_g[:, p : p + 1],
                    gate_f[:, p : p + 1], ALU.mult,
                )
                if last:
                    # plane 5's t total arrives as two chunk sums
                    nc.vector.tensor_tensor(
                        D2[:, p : p + 1], TT_t[:, p : p + 1],
                        TT_t[:, p + 1 : p + 2], ALU.add,
                    )
                    nc.vector.tensor_tensor(
                        D2[:, p : p + 1], D2[:, p : p + 1],
                        gate_f[:, PLANES + p : PLANES + p + 1], ALU.mult,
                    )
                else:
                    nc.vector.tensor_tensor(
                        D2[:, p : p + 1], TT_t[:, p : p + 1],
                        gate_f[:, PLANES + p : PLANES + p + 1], ALU.mult,
                    )
                nc.vector.tensor_tensor(
                    D1[:, p : p + 1], D1[:, p : p + 1], D2[:, p : p + 1], ALU.add
                )
                nc.scalar.activation(
                    A[:, p : p + 1], D1[:, p : p + 1], AF.Sigmoid,
                    bias=gate_f[:, 2 * PLANES + p : 2 * PLANES + p + 1],
                )
                nc.vector.tensor_scalar(
                    B[:, p : p + 1], A[:, p : p + 1], -1.0, 1.0, ALU.mult, ALU.add
                )
                nc.vector.tensor_scalar(
                    diagA[p][:], ident_f[:], A[:, p : p + 1], None, ALU.mult
                )
                nc.vector.tensor_scalar(
                    diagB[p][:], ident_f[:], B[:, p : p + 1], None, ALU.mult
                )

            def mms(p):
                # z = a*g + b*t - s accumulated in PSUM by TensorE.
                # Two half-plane PSUM tiles (2 banks each) so the ACT-drained
                # and DVE-drained halves recycle independently.
                za = psp.tile([P, PF // 2], f32, name=f"za{p}", tag="za")
                zb = psp.tile([P, PF // 2], f32, name=f"zb{p}", tag="zb")
                zps[p] = (za, zb)
                gs, ss = g_sl(p), s_sl(p)
                for ci in range(PF // CHUNK):
                    zp = za if ci < (PF // CHUNK) // 2 else zb
                    sl = slice((ci % 2) * CHUNK, (ci % 2) * CHUNK + CHUNK)
                    gsl = slice(ci * CHUNK, (ci + 1) * CHUNK)
                    nc.tensor.matmul(
                        zp[:, sl], diagA[p][:], gs[:, gsl], start=True, stop=False
                    )
                    nc.tensor.matmul(
                        zp[:, sl], diagB[p][:], t_chunk(p, ci), start=False, stop=False
                    )
                    nc.tensor.matmul(
                        zp[:, sl], negi_r[:], ss[:, gsl], start=False, stop=True
                    )

            def drain(p):
                za, zb = zps[p]
                # abs written back in place to PSUM: no SBUF scratch, so no
                # WAW chain against phase-1's scratch dumps on ACT
                nc.scalar.activation(
                    za[:], za[:], AF.Abs,
                    accum_out=R_act[:, p : p + 1],
                )
                nc.vector.tensor_reduce(
                    R_dve[:, p : p + 1], zb[:],
                    mybir.AxisListType.X, ALU.add, apply_absolute_value=True,
                )

            # half 0: s0p2's DMA is emitted after gate(1) so PAR(0)/PAR(1)
            # slot into Pool's ring between the s-piece transfers.
            for p in range(HALFP):
                ph1_g(p)
                ph1_t(p)
                gate(p)
                mms(p)
            # half-1 tail pieces (after the half-0 PARs in Pool's stream):
            # t pieces ride the SP ring (their chain is long: sum->gate->mm),
            # s pieces ride the Pool ring interleaved with the half-1 PARs.
            nc.sync.dma_start(out=t1p[0][:], in_=tgt_e[1, :, 0:PF])
            nc.gpsimd.dma_start(out=s1p[0][:], in_=sgt_e[1, :, 0:PF])
            nc.sync.dma_start(out=t1p[1][:], in_=tgt_e[1, :, PF : 2 * PF])
            nc.gpsimd.dma_start(out=s1p[1][:], in_=sgt_e[1, :, PF : 2 * PF])
            nc.sync.dma_start(out=t1p2a[:], in_=tgt_e[1, :, 2 * PF : 2 * PF + PF // 2])
            nc.sync.dma_start(out=t1p2b[:], in_=tgt_e[1, :, 2 * PF + PF // 2 : 3 * PF])
            nc.gpsimd.dma_start(out=s1p[2][:], in_=sgt_e[1, :, 2 * PF : 3 * PF])
            drain(0)
            drain(1)
            # half 1: g sums hoisted (g1 lands before the t/s pieces); per-
            # plane chains follow each t piece; drains staggered behind.
            ph1_g(3)
            ph1_g(4)
            ph1_g(5)
            ph1_t(3)
            gate(3)
            mms(3)
            drain(2)
            ph1_t(4)
            gate(4)
            mms(4)
            ph1_t(5)
            gate(5)
            mms(5)
            drain(3)
            drain(4)
            drain(5)

            # collapse partitions so the output DMA is one descriptor
            nc.gpsimd.partition_all_reduce(
                R2a[:], R_act[:], channels=P, reduce_op=bass_isa.ReduceOp.add
            )
            nc.gpsimd.partition_all_reduce(
                R2b[:], R_dve[:], channels=P, reduce_op=bass_isa.ReduceOp.add
            )
            nc.sync.dma_start(out=out_e[0:1, 0:PLANES], in_=R2a[0:1, :])
            nc.sync.dma_start(out=out_e[0:1, PLANES : 2 * PLANES], in_=R2b[0:1, :])

    nc.finalize()
    return nc


def _gate_matrix(convW, convB, linW, linB):
    """Fold the per-channel 2->32->2 MLP + softmax-diff into d = alpha.pooled+beta."""
    w = (linW[:, 0, :] - linW[:, 1, :]).astype(np.float64)        # [3,32]
    alpha = np.einsum("co,coj->cj", w, convW.astype(np.float64))  # [3,2]
    beta = (w * convB.astype(np.float64)).sum(1) + (
        linB[:, 0].astype(np.float64) - linB[:, 1].astype(np.float64)
    )                                                             # [3]
    row = np.zeros(3 * PLANES, dtype=np.float64)
    for p in range(PLANES):
        c = p % C
        row[p] = alpha[c, 0] / HWC
        row[PLANES + p] = alpha[c, 1] / HWC
        row[2 * PLANES + p] = beta[c]
    return row.astype(np.float32)


def _make_in_maps(inputs):
    gate_row = _gate_matrix(
        np.asarray(inputs["convW"], dtype=np.float32),
        np.asarray(inputs["convB"], dtype=np.float32),
        np.asarray(inputs["linW"], dtype=np.float32),
        np.asarray(inputs["linB"], dtype=np.float32),
    )
    consts = np.zeros((P, NCONST), dtype=np.float32)
    consts[:, 0 : 3 * PLANES] = gate_row[None, :]
    consts[:, 3 * PLANES : 3 * PLANES + P] = np.eye(P, dtype=np.float32)
    consts[:, 3 * PLANES + P : NCONST] = -np.eye(P, dtype=np.float32)

    def shards(x):
        x = np.asarray(x, dtype=np.float32)
        # [16,3,512,512] -> per core [2 halves, 128, 3*2048]
        x = x.reshape(NCORES, 2, HALFP, P, PF)
        x = x.transpose(0, 1, 3, 2, 4)  # [cores, half, P, planes, PF]
        return np.ascontiguousarray(x.reshape(NCORES, 2, P, HF))

    g_s, t_s, s_s = shards(inputs["gt"]), shards(inputs["t_gt"]), shards(inputs["s_gt"])
    return [
        {"gt": g_s[i], "t_gt": t_s[i], "s_gt": s_s[i], "consts": consts}
        for i in range(NCORES)
    ]


def _run(inputs, trace=False):
    import time

    from concourse.bass_utils import run_bass_kernel_spmd

    if "nc" not in _CACHE:
        _CACHE["nc"] = _build_nc()
    nc = _CACHE["nc"]

    in_maps = _make_in_maps(inputs)
    res = None
    for attempt in range(3):
        try:
            res = run_bass_kernel_spmd(nc, in_maps, list(range(NCORES)), trace=trace)
            break
        except Exception:
            # first execution of a freshly compiled NEFF occasionally hits a
            # transient device error on this fleet; retry
            if attempt == 2:
                raise
            time.sleep(10)
    total = np.float64(0.0)
    for i in range(NCORES):
        total += np.asarray(res.results[i]["out"], dtype=np.float64).sum()
    mean = total / float(N * C * H * W)
    return np.float32(LOSS_WEIGHT * mean), res


def kernel(**inputs) -> np.ndarray:
    out, _ = _run(inputs, trace=False)
    return out
